# revision 9
# baseline (speedup 1.0000x reference)
"""Trainium2 Bass kernel for nn_EquivariantDecoder (GNN message passing).

Sharding: nodes are split into 8 contiguous ranges of 6272 (= 49 tiles of
128); each core owns the edges whose dst lands in its range, so per-node
segment sums are core-local (no collectives).

Schedule: each core sorts its 49 node-groups by edge-tile count
descending; slot j gets K_j = max over cores of the j-th largest
per-group tile count, so one SPMD program covers all cores with ~2% edge
padding (host un-permutes the per-slot geom output at the end).

Edge tiles stream in units of 12 (one DMA + one [128,1536] activation per
MLP half per unit -- the act bias differs per half, which caps act width
at one half). Per unit:
  mm1: 6 fp8 DoubleRow matmuls (inputs pre-scaled x16 on host)
  s1 = silu(z + b1)                      (scalar engine, 2 acts, bf16)
  u1 = s1_0*w2_0 + s1_1*w2_1             (DVE: TS, TS, TT)
  psw = colsum(u1) via ones-rhs matmuls  (PE, 1 per tile)
  msg = (psw + b2) * rel                 (DVE STT, bf16)
  scatter: flipped one-hot matmul: lhsT = msg [128e, 3], rhs = host-built
  one-hot [128e, 128n] fp8 -> psum [3, 128n] accumulated per slot; banks
  of 4 slots copied to SBUF, one geo DMA at the end.
Node path (independent of scatter): alpha = silu(h @ vgW1 + vgb1) @ vgW2,
6 node tiles per group; out = sum_k alpha_k * vel_k. The vgb2 bias term
and the geom mean+add are folded into the host finalize.
"""

import sys

import numpy as np

try:
    import concourse.bass as bass  # noqa: F401
except Exception:  # pragma: no cover
    sys.path.insert(0, "/opt/trn_rl_repo")

import concourse.bass as bass
import concourse.mybir as mybir
from concourse.bass_utils import run_bass_kernel_spmd
from concourse.tile import TileContext
from concourse.vector_clock import ScopedClock

N_NODES = 50000
N_EDGES = 800000
H = 256
N_CORES = 8
NT = 49                 # node tiles (=groups) per core
NPC = NT * 128          # 6272 nodes per core
N_PAD = N_CORES * NPC   # 50176
P = 128
U = 12                  # edge tiles per streaming unit
NG = 6                  # node tiles batched per node-path group

BF16 = mybir.dt.bfloat16
BF16_NP = mybir.dt.np(BF16)
F8 = mybir.dt.float8e4
F8_NP = mybir.dt.np(F8)
F32 = mybir.dt.float32
AF = mybir.ActivationFunctionType
AF_USED = AF.Silu   # sim tests may override (interp lacks Silu)
OP = mybir.AluOpType
DR = mybir.MatmulPerfMode.DoubleRow
MM_SCALE = 16.0         # host pre-scale on m_ij and W1 (undone by 1/256)


# ---------------------------------------------------------------------------
# Walrus on this toolchain rejects >2 sync waits on the TileContext tail
# drain ("Too many sync wait commands"); split them across SP NOPs.
def _patched_drain_and_barrier(self, tick_clock, wait_clock):
    drain_inst = self.nc.sync.drain()
    wait_clock.add_sem_waits(
        drain_inst.ins, ScopedClock({None: tick_clock.global_clock})
    )
    si = drain_inst.ins.sync_info
    if si is not None and si.on_wait and len(si.on_wait) > 1:
        extra = list(si.on_wait[1:])
        del si.on_wait[1:]
        for w in extra:
            nop = self.nc.sync.nop(nofuse=True, hint="drain_wait_split")
            nsi = nop.ins.sync_info
            if nsi is None:
                nop.ins.sync_info = mybir.SyncInfo(on_wait=[w], on_update=[])
            else:
                nsi.on_wait.append(w)

    self.nc.all_engine_barrier()
    assert self.sems is not None
    popped = self.nc._tile_sem_poison_stack.pop()
    assert popped is self._sem_poison
    self.nc.clear_and_free_semaphores(list(self.sems.allocated().values()))
    self.nc.all_engine_barrier()


TileContext._drain_and_barrier = _patched_drain_and_barrier


def _split_excess_waits(nc, maxw: int = 1):
    """Walrus rejects >maxw sync waits on one instruction; move the excess
    onto NOPs inserted just before, on the same engine (same-queue program
    order makes this equivalent)."""
    n_split = 0
    for f in nc.m.functions:
        for b in f.blocks:
            out = []
            for inst in b.instructions:
                si = inst.sync_info
                if si is not None and si.on_wait and len(si.on_wait) > maxw:
                    extra = list(si.on_wait[: -maxw])
                    del si.on_wait[: -maxw]
                    for i in range(0, len(extra), maxw):
                        nop = mybir.InstNoOp(
                            name=f"{inst.name}-wsplit{i}",
                            engine=inst.engine,
                            sync_info=mybir.SyncInfo(
                                on_wait=extra[i:i + maxw], on_update=[]),
                            bass_nofuse=True,
                        )
                        out.append(nop)
                    n_split += 1
                out.append(inst)
            b.instructions[:] = out
    return n_split
# ---------------------------------------------------------------------------


def _build_program(Ks: list[int], b2: float):
    """Trace the single-core SPMD program. Ks[j] = edge tiles in schedule
    slot j (shared across cores); sum(Ks) is a multiple of U."""
    ET = sum(Ks)                     # edge tiles per core
    n_unit = ET // U

    # tile -> (slot, idx-in-slot, slot-size)
    slot_of = []
    for j, K in enumerate(Ks):
        for i in range(K):
            slot_of.append((j, i, K))

    nc = bass.Bass()

    mijT = nc.dram_tensor("mijT", [n_unit, P, U * 2 * P], F8,
                          kind="ExternalInput")
    ohT = nc.dram_tensor("ohT", [n_unit, P, U * P], F8, kind="ExternalInput")
    rel_d = nc.dram_tensor("rel", [P, ET * 3], F32, kind="ExternalInput")
    hT = nc.dram_tensor("hT", [NT, P, 2 * P], BF16, kind="ExternalInput")
    velg_d = nc.dram_tensor("velg", [P, NT * 15], F32, kind="ExternalInput")
    w1dr_d = nc.dram_tensor("w1dr", [2, P, 2 * P], F8, kind="ExternalInput")
    w2c_d = nc.dram_tensor("w2c", [2, P, 1], F32, kind="ExternalInput")
    b1t_d = nc.dram_tensor("b1t", [2, P, 1], F32, kind="ExternalInput")
    vgw1b_d = nc.dram_tensor("vgw1b", [2, 2, P, P], BF16, kind="ExternalInput")
    vgw2t_d = nc.dram_tensor("vgw2t", [2, P, 5], BF16, kind="ExternalInput")
    vgb1t_d = nc.dram_tensor("vgb1t", [2, P, 1], F32, kind="ExternalInput")
    onesp_d = nc.dram_tensor("onesp", [P, 1], BF16, kind="ExternalInput")
    out_d = nc.dram_tensor("out", [P, NT * 3], F32, kind="ExternalOutput")
    geo_d = nc.dram_tensor("geo", [3, NT * P], F32, kind="ExternalOutput")

    with TileContext(nc) as tc:
        with (
            tc.tile_pool(name="const", bufs=1) as cpool,
            tc.tile_pool(name="rhs", bufs=4) as rhs_pool,
            tc.tile_pool(name="s1", bufs=4) as s1_pool,
            tc.tile_pool(name="small", bufs=6) as sm_pool,
            tc.tile_pool(name="oh", bufs=6) as oh_pool,
            tc.tile_pool(name="nodes", bufs=3) as nd_pool,
            tc.tile_pool(name="ps_mm1", bufs=2, space="PSUM") as ps1_pool,
            tc.tile_pool(name="ps_w", bufs=1, space="PSUM") as psw_pool,
            tc.tile_pool(name="ps_geo", bufs=1, space="PSUM") as psg_pool,
        ):
            # ---- edge-path streaming prefetch (issued FIRST so the first
            # units' mm1 inputs are not queued behind the big const DMAs)
            sup_t = {}
            oh_t = {}

            def prefetch(u):
                st = rhs_pool.tile([P, U * 2 * P], F8, tag="sup", name="sup")
                nc.sync.dma_start(st[:], mijT[u, :, :])
                ot = oh_pool.tile([P, U * P], F8, tag="oh", name="ohs")
                nc.sync.dma_start(ot[:], ohT[u, :, :])
                sup_t[u] = st
                oh_t[u] = ot

            w1 = [cpool.tile([P, 2 * P], F8, tag=f"w1_{hh}", name=f"w1_{hh}")
                  for hh in range(2)]
            for hh in range(2):
                nc.sync.dma_start(w1[hh][:], w1dr_d[hh, :, :])
            for u in range(min(3, n_unit)):
                prefetch(u)

            vgw1 = [[cpool.tile([P, P], BF16, tag=f"vgw1_{kk}{hh}",
                                name=f"vgw1_{kk}{hh}")
                     for hh in range(2)] for kk in range(2)]
            for kk in range(2):
                for hh in range(2):
                    nc.gpsimd.dma_start(vgw1[kk][hh][:], vgw1b_d[kk, hh, :, :])
            w2c = [cpool.tile([P, 1], F32, tag=f"w2c_{hh}", name=f"w2c_{hh}")
                   for hh in range(2)]
            b1 = [cpool.tile([P, 1], F32, tag=f"b1_{hh}", name=f"b1_{hh}")
                  for hh in range(2)]
            vgw2 = [cpool.tile([P, 5], BF16, tag=f"vgw2_{hh}", name=f"vgw2_{hh}")
                    for hh in range(2)]
            vgb1 = [cpool.tile([P, 1], F32, tag=f"vgb1_{hh}", name=f"vgb1_{hh}")
                    for hh in range(2)]
            for hh in range(2):
                nc.scalar.dma_start(w2c[hh][:], w2c_d[hh, :, :])
                nc.scalar.dma_start(b1[hh][:], b1t_d[hh, :, :])
                nc.gpsimd.dma_start(vgw2[hh][:], vgw2t_d[hh, :, :])
                nc.gpsimd.dma_start(vgb1[hh][:], vgb1t_d[hh, :, :])
            onesp = cpool.tile([P, 1], BF16, tag="onesp")
            nc.scalar.dma_start(onesp[:], onesp_d[:, :])

            rel = cpool.tile([P, ET * 3], F32, tag="rel")
            nc.scalar.dma_start(rel[:], rel_d[:, :])
            velg = cpool.tile([P, NT * 15], F32, tag="velg")
            nc.gpsimd.dma_start(velg[:], velg_d[:, :])

            # all node features resident in SBUF: one DMA, no per-group loads
            hTall = cpool.tile([P, NT * 2 * P], BF16, tag="hTall")
            nc.gpsimd.dma_start(
                hTall[:].rearrange("p (t c) -> p t c", t=NT),
                hT[:, :, :].rearrange("t p c -> p t c"))

            # packed outputs; single DMA each at the end
            outbuf = cpool.tile([P, NT * 3], F32, tag="outbuf")
            geomb = cpool.tile([3, NT * P], F32, tag="geomb")

            relv = rel.rearrange("p (t f) -> p t f", f=3)
            rhv = hTall.rearrange("p (t kk n) -> p t kk n", kk=2, n=P)

            def node_group(g0: int, T: int):
                """Node tiles g0..g0+T-1: vel-gate MLP + vel combine."""
                psn = ps1_pool.tile([P, 2 * NG * P], F32, tag="ps_mm1",
                                    name="psn")
                # 2-tile (256-col) matmul chunks keep every PSUM write
                # within a single 512-fp32 bank (hh1 starts at offset 768)
                n_nch = (T + 1) // 2
                for hh in range(2):
                    for ch in range(n_nch):
                        tl, th = ch * 2, min(T, ch * 2 + 2)
                        for kk in range(2):
                            nc.tensor.matmul(
                                psn[:, hh * NG * P + tl * P:
                                    hh * NG * P + th * P],
                                vgw1[kk][hh][:],
                                rhv[:, g0 + tl:g0 + th, kk, :],
                                start=(kk == 0), stop=(kk == 1))
                s1n = [nd_pool.tile([P, NG * P], BF16, tag=f"s1n_{hh}",
                                    name=f"s1n_{hh}")
                       for hh in range(2)]
                for hh in range(2):
                    nc.scalar.activation(s1n[hh][:, 0:T * P],
                                         psn[:, hh * NG * P:
                                             hh * NG * P + T * P],
                                         AF_USED,
                                         bias=vgb1[hh][:, 0:1], scale=1.0)
                for t in range(T):
                    nt = g0 + t
                    psa = psw_pool.tile([P, 8], F32, tag="ps_w", name="psa")
                    for hh in range(2):
                        nc.tensor.matmul(psa[:, 0:5],
                                         s1n[hh][:, t * P:(t + 1) * P],
                                         vgw2[hh][:],
                                         start=(hh == 0), stop=(hh == 1))

                    # out[:, j] = sum_k alpha[k] * vel[j, k]
                    scratch = sm_pool.tile([P, 15], F32, tag="scratch")
                    vbase = nt * 15
                    velg_v = velg[:, vbase:vbase + 15].rearrange(
                        "p (j k) -> p j k", k=5)
                    nc.vector.tensor_tensor(
                        scratch[:].rearrange("p (j k) -> p j k", k=5),
                        velg_v,
                        psa[:, None, 0:5].broadcast_to([P, 3, 5]),
                        op=OP.mult)
                    nc.vector.tensor_reduce(
                        outbuf[:, nt * 3:nt * 3 + 3, None],
                        scratch[:].rearrange("p (j k) -> p j k", k=5),
                        axis=mybir.AxisListType.X, op=OP.add)

            # ---- software-pipelined edge-path unit loop -------------------
            # PE program order per iteration u:
            #   mm1(u), u-mm(u-1), scatter(u-2)
            # so the PE never waits on the scalar/DVE chain of the current
            # unit and stays continuously busy (HAM stays at K=8/8).
            w1v = [w1[hh].rearrange("p (kk m) -> p kk m", kk=2)
                   for hh in range(2)]
            stA = {}   # u -> state for u-mm+msg stage
            stB = {}   # u -> state for scatter stage
            geo_ref = [None]

            def head(u):
                t0 = u * U
                G = min(U, ET - t0)          # real edge tiles in this unit
                W = G * P                    # unit width in edges
                if u + 3 < n_unit:
                    prefetch(u + 3)
                sup = sup_t.pop(u)
                ohs = oh_t.pop(u)
                supv = sup.rearrange("p (kk e) -> p kk e", kk=2)
                ps1 = [ps1_pool.tile([P, U * P], F32, tag="ps_mm1", name="ps1")
                       for _ in range(2)]
                n_ch = (W + 511) // 512
                for hh in range(2):
                    for ch in range(n_ch):
                        cw = min(512, W - ch * 512)
                        nc.tensor.matmul(
                            ps1[hh][:, ch * 512:ch * 512 + cw],
                            w1v[hh],
                            supv[:, :, ch * 512:ch * 512 + cw],
                            start=True, stop=True, perf_mode=DR)
                s1 = [s1_pool.tile([P, U * P], BF16, tag=f"s1_{hh}",
                                   name=f"s1_{hh}")
                      for hh in range(2)]
                for hh in range(2):
                    nc.scalar.activation(s1[hh][:, 0:W], ps1[hh][:, 0:W],
                                         AF_USED,
                                         bias=b1[hh][:, 0:1],
                                         scale=1.0 / (MM_SCALE * MM_SCALE))
                # u = s1_0*w2_0 + s1_1*w2_1  (DVE; tensor_scalar runs 4x on
                # bf16 SBUF, tensor_tensor 2x; scalar_tensor_tensor is 1x)
                u0 = s1_pool.tile([P, U * P], BF16, tag="u0", name="u0")
                nc.vector.tensor_scalar(u0[:, 0:W], s1[0][:, 0:W],
                                        w2c[0][:, 0:1], None, op0=OP.mult)
                t1 = s1_pool.tile([P, U * P], BF16, tag="t1", name="t1")
                nc.vector.tensor_scalar(t1[:, 0:W], s1[1][:, 0:W],
                                        w2c[1][:, 0:1], None, op0=OP.mult)
                u1 = s1_pool.tile([P, U * P], BF16, tag="u1", name="u1")
                nc.vector.tensor_add(u1[:, 0:W], t1[:, 0:W], u0[:, 0:W])
                stA[u] = (u1, G, t0, ohs)

            def stage_umm(u):
                u1, G, t0, ohs = stA.pop(u)
                # per-tile w sums: psw[:, c] = sum_h u1[h, tile c]
                psw = psw_pool.tile([P, U], F32, tag="ps_w")
                for c in range(G):
                    nc.tensor.matmul(psw[:, c:c + 1],
                                     u1[:, c * P:(c + 1) * P],
                                     onesp[:], start=True, stop=True)
                # msg for all tiles in one op: [P, G, 3] = (w + b2) * rel
                msg = sm_pool.tile([P, U * 3], BF16, tag="msg")
                nc.vector.scalar_tensor_tensor(
                    msg[:, 0:3 * G].rearrange("p (c f) -> p c f", f=3),
                    psw[:, 0:G, None].broadcast_to([P, G, 3]),
                    float(b2),
                    relv[:, t0:t0 + G, 0:3],
                    op0=OP.add, op1=OP.mult)
                stB[u] = (msg, G, t0, ohs)

            def stage_scatter(u):
                msg, G, t0, ohs = stB.pop(u)
                for c in range(G):
                    t = t0 + c
                    j, i, K = slot_of[t]
                    if j % 4 == 0 and i == 0:
                        geo_ref[0] = psg_pool.tile(
                            [3, 512], F32, tag="ps_geo", name="geo")
                    ohc = ohs[:, c * P:(c + 1) * P]
                    off = (j % 4) * P
                    nc.tensor.matmul(geo_ref[0][0:3, off:off + P],
                                     msg[:, 3 * c:3 * c + 3],
                                     ohc,
                                     start=(i == 0), stop=(i == K - 1))
                    if i == K - 1 and (j % 4 == 3 or j == NT - 1):
                        lo = (j // 4) * 512
                        wdt = (j % 4) * P + P
                        nc.vector.tensor_copy(geomb[0:3, lo:lo + wdt],
                                              geo_ref[0][0:3, 0:wdt])

            # node groups spread through the first ~3/4 of the unit loop so
            # the tail drains fast
            n_grp = (NT + NG - 1) // NG
            span = max(n_grp, (n_unit * 3) // 4)
            trig = {max(2, ((idx + 1) * span) // n_grp): idx
                    for idx in range(n_grp)}

            for u in range(n_unit + 2):
                if u < n_unit:
                    head(u)
                if u >= 1 and u - 1 < n_unit:
                    stage_umm(u - 1)
                if u >= 2:
                    stage_scatter(u - 2)
                if u in trig:
                    g0 = trig[u] * NG
                    node_group(g0, min(NG, NT - g0))

            nc.sync.dma_start(out_d[:, :], outbuf[:])
            nc.sync.dma_start(geo_d[:, :], geomb[0:3, :])

    _split_excess_waits(nc)
    return nc


def _preprocess(inputs: dict):
    """Shard + lay out all per-core device inputs.

    Returns (in_maps, Ks, b2, perms, invc, velbias)."""
    h = np.asarray(inputs["h"], np.float32)
    m_ij = np.asarray(inputs["m_ij"], np.float32)
    x = np.asarray(inputs["x"], np.float32)
    vel_all = np.asarray(inputs["vel_all"], np.float32)
    ei = np.asarray(inputs["edge_index"])
    src = ei[0].astype(np.int64)
    dst = ei[1].astype(np.int64)

    counts = np.bincount(dst, minlength=N_NODES).astype(np.float32)
    invc = (1.0 / np.maximum(counts, 1.0)).astype(np.float32)

    order = np.argsort(dst, kind="stable")
    dst_s = dst[order]
    src_s = src[order]
    rel_s = (x[src_s] - x[dst_s]).astype(np.float32)
    mij_s = (m_ij[order] * MM_SCALE).astype(F8_NP)

    g_all = dst_s // P                   # global 128-node group, 0..391
    n_groups = N_CORES * NT
    cg = np.bincount(g_all, minlength=n_groups)
    gstart = np.zeros(n_groups + 1, np.int64)
    gstart[1:] = np.cumsum(cg)
    tiles_needed = -(-cg // P)           # ceil, [392]
    tn = tiles_needed.reshape(N_CORES, NT)

    # schedule: per core, sort its groups by tile count desc; slot j's
    # size is the max over cores (shared SPMD program structure).
    perms = [np.argsort(-tn[k], kind="stable") for k in range(N_CORES)]
    Ks = np.max(np.stack([tn[k][perms[k]] for k in range(N_CORES)]), axis=0)
    Ks = np.maximum(Ks.astype(np.int64), 1)
    ET = int(Ks.sum())
    pad = (-ET) % U
    Ks[-1] += pad                        # pad tiles ride in the last slot
    ET += pad
    Ks = [int(v) for v in Ks]
    kstart = np.zeros(NT + 1, np.int64)
    kstart[1:] = np.cumsum(Ks)

    # weights (shared by all cores)
    w1 = np.asarray(inputs["ew_W1"], np.float32)
    b1 = np.asarray(inputs["ew_b1"], np.float32)
    w2 = np.asarray(inputs["ew_W2"], np.float32)
    b2 = float(np.asarray(inputs["ew_b2"], np.float32)[0])
    vgw1 = np.asarray(inputs["vg_W1"], np.float32)
    vgb1 = np.asarray(inputs["vg_b1"], np.float32)
    vgw2 = np.asarray(inputs["vg_W2"], np.float32)
    vgb2 = np.asarray(inputs["vg_b2"], np.float32)

    # w1dr[hh][p, kk*128+m] = 16*W1[kk*128+p, hh*128+m]  (DoubleRow layout)
    w1s = (w1 * MM_SCALE).reshape(2, P, 2, P)            # [kk, p, hh, m]
    w1dr = np.ascontiguousarray(
        w1s.transpose(2, 1, 0, 3)).reshape(2, P, 2 * P).astype(F8_NP)
    w2c = w2.reshape(2, P, 1).astype(np.float32).copy()
    b1t = b1.reshape(2, P, 1).copy()
    vgw1b = vgw1.reshape(2, P, 2, P).transpose(0, 2, 1, 3).astype(BF16_NP).copy()
    vgw2t = vgw2.reshape(2, P, 5).astype(BF16_NP).copy()
    vgb1t = vgb1.reshape(2, P, 1).copy()
    onesp = np.ones((P, 1), BF16_NP)

    # padded node tensors
    hp = np.zeros((N_PAD, H), np.float32)
    hp[:N_NODES] = h
    velp = np.zeros((N_PAD, 5, 3), np.float32)
    velp[:N_NODES] = vel_all
    velbias = np.einsum("nkf,k->nf", velp, vgb2).astype(np.float32)

    n_unit = ET // U
    S = ET * P                           # edge slots per core

    in_maps = []
    for k in range(N_CORES):
        perm = perms[k]
        # gather this core's edges into slot order
        mijp = np.zeros((S, H), F8_NP)
        relp = np.zeros((S, 3), np.float32)
        colidx = np.full(S, -1, np.int64)
        for j in range(NT):
            g_local = int(perm[j])
            g = k * NT + g_local
            e0, e1 = int(gstart[g]), int(gstart[g + 1])
            s0 = int(kstart[j]) * P
            mijp[s0:s0 + e1 - e0] = mij_s[e0:e1]
            relp[s0:s0 + e1 - e0] = rel_s[e0:e1]
            colidx[s0:s0 + e1 - e0] = dst_s[e0:e1] - (g * P)

        mv = mijp.reshape(ET, P, H)
        rv = relp.reshape(ET, P, 3)
        cv = colidx.reshape(ET, P)

        # mijT units: [n_unit, 128, U*256]; free = (kk, tile, e),
        # partition = h within kk half
        b = mv.transpose(0, 2, 1).reshape(ET, 2, P, P)   # [t, kk, p, e]
        mijT = np.ascontiguousarray(
            b.reshape(n_unit, U, 2, P, P).transpose(0, 3, 2, 1, 4)
        ).reshape(n_unit, P, U * 2 * P)

        rel = np.ascontiguousarray(
            rv.transpose(1, 0, 2)).reshape(P, ET * 3)

        # host-built one-hot: ohT[unit][p, (tile, n)] = (col[t, p] == n)
        oh = (cv[:, :, None] ==
              np.arange(P, dtype=np.int64)[None, None, :])  # [ET, P, 128]
        ohT = np.ascontiguousarray(
            oh.astype(F8_NP).reshape(n_unit, U, P, P).transpose(0, 2, 1, 3)
        ).reshape(n_unit, P, U * P)

        hk = hp[k * NPC:(k + 1) * NPC].reshape(NT, P, H)
        hTk = np.ascontiguousarray(
            hk.transpose(0, 2, 1).reshape(NT, 2, P, P).transpose(0, 2, 1, 3)
        ).reshape(NT, P, 2 * P).astype(BF16_NP)

        # velg cols per node tile: [comp j, gate k] at 5*j+k
        vg = (velp[k * NPC:(k + 1) * NPC]
              .reshape(NT, P, 5, 3).transpose(1, 0, 3, 2)
              .reshape(P, NT * 15))
        velg = np.ascontiguousarray(vg)

        in_maps.append({
            "mijT": mijT,
            "ohT": ohT,
            "rel": rel,
            "hT": hTk,
            "velg": velg,
            "w1dr": w1dr,
            "w2c": w2c,
            "b1t": b1t,
            "vgw1b": vgw1b,
            "vgw2t": vgw2t,
            "vgb1t": vgb1t,
            "onesp": onesp,
        })
    return in_maps, Ks, b2, perms, invc, velbias


def unpack_out(arr: np.ndarray) -> np.ndarray:
    """[128, NT*3] packed per-core output -> [NPC, 3]."""
    return arr.reshape(P, NT, 3).transpose(1, 0, 2).reshape(NPC, 3)


def finalize(results, perms, invc, velbias) -> np.ndarray:
    """Combine per-core vel output + slot-ordered geom sums on the host."""
    out = np.empty((N_PAD, 3), np.float32)
    for k in range(N_CORES):
        vel = unpack_out(np.asarray(results[k]["out"], np.float32))
        geo = np.asarray(results[k]["geo"], np.float32)  # [3, NT*128]
        geoT = geo.T.reshape(NT, P, 3)
        geom = np.empty((NPC, 3), np.float32)
        perm = perms[k]
        for j in range(NT):
            geom[perm[j] * P:(perm[j] + 1) * P] = geoT[j]
        nodes = slice(k * NPC, (k + 1) * NPC)
        iv = np.ones(NPC, np.float32)
        n_real = min(N_NODES - k * NPC, NPC)
        if n_real > 0:
            iv[:n_real] = invc[k * NPC:k * NPC + n_real]
        out[nodes] = vel + velbias[nodes] + geom * iv[:, None]
    return out[:N_NODES]


def kernel(**inputs) -> np.ndarray:
    in_maps, Ks, b2, perms, invc, velbias = _preprocess(inputs)
    nc = _build_program(Ks, b2)
    res = run_bass_kernel_spmd(nc, in_maps, list(range(N_CORES)))
    return finalize(res.results, perms, invc, velbias).astype(np.float32)
